# revision 4
# baseline (speedup 1.0000x reference)
"""Two-layer GCN (PyG GCNConv x2 + ReLU) on 8 Trainium2 NeuronCores.

Strategy: nodes are packed into 128-slot tiles balanced by in-degree and
sharded across cores (49 tiles/core). Each core redundantly computes the
full g1 = dinv * (x @ W1) gather table from a per-core ROTATED copy of x^T
(own nodes at rows 0..6271, so all SPMD addressing is static). Aggregation
is dma_gather of source rows + one-hot (is_equal) matmuls accumulating in
PSUM; self-loops are an identity matmul. Layer-2 input g2 = dinv * (a1 @ W2)
is computed fused per tile and exchanged with a chunked AllGather.
"""

import math
import heapq

import numpy as np
import ml_dtypes

from concourse import bacc, mybir
from concourse.tile import TileContext
from concourse.bass_utils import run_bass_kernel_spmd

BF16 = ml_dtypes.bfloat16
N_CORES = 8

# Full-problem config. Tests may monkeypatch _CFG before calling kernel().
_CFG = dict(
    N=50000,
    E=800000,
    IN=768,
    HID=512,
    OUT=256,
    T=49,  # tiles per core
)


def _pack_idx(idx_linear):
    """[K*128] int -> [128, K*8] int16 in dma_gather layout."""
    num = idx_linear.shape[0]
    a = idx_linear.reshape(num // 16, 16).T.astype(np.int16)
    return np.tile(a, (8, 1))


def _build_nc(cfg, meta):
    IN, HID, OUT = cfg["IN"], cfg["HID"], cfg["OUT"]
    T = cfg["T"]
    PC = T * 128
    NPAD = PC * N_CORES
    TT = T * N_CORES
    HALF = NPAD // 2
    KL1, KH1, KL2, KH2 = meta["KL1"], meta["KH1"], meta["KL2"], meta["KH2"]
    NK1 = IN // 128
    NK2 = HID // 128
    # AllGather chunks (in tiles per core)
    ch_tiles = meta["ch_tiles"]

    f32 = mybir.dt.float32
    bf = mybir.dt.bfloat16
    i16 = mybir.dt.int16

    nc = bacc.Bacc(None, target_bir_lowering=False, debug=False)
    xT_p = nc.declare_dram_parameter("xT", [IN, NPAD], bf, isOutput=False)
    w1_p = nc.declare_dram_parameter("w1p", [128, NK1 * HID], bf, isOutput=False)
    w2_p = nc.declare_dram_parameter("w2p", [128, NK2 * OUT], bf, isOutput=False)
    b1_p = nc.declare_dram_parameter("b1r", [128, HID], f32, isOutput=False)
    b2_p = nc.declare_dram_parameter("b2r", [128, OUT], f32, isOutput=False)
    iota_p = nc.declare_dram_parameter("iota", [128, 128], f32, isOutput=False)
    ident_p = nc.declare_dram_parameter("identb", [128, 128], bf, isOutput=False)
    dinv_p = nc.declare_dram_parameter("dinvT", [128, TT], f32, isOutput=False)
    idx1_p = nc.declare_dram_parameter("idx1", [T * 128, (KL1 + KH1) * 8], i16, isOutput=False)
    dl1_p = nc.declare_dram_parameter("dl1", [T * 128, KL1 + KH1], f32, isOutput=False)
    idx2_p = nc.declare_dram_parameter("idx2", [T * 128, (KL2 + KH2) * 8], i16, isOutput=False)
    dl2_p = nc.declare_dram_parameter("dl2", [T * 128, KL2 + KH2], f32, isOutput=False)
    out_p = nc.declare_dram_parameter("out", [PC, OUT], f32, isOutput=True)

    with TileContext(nc) as tc:
        with (
            tc.tile_pool(name="const", bufs=1) as cpool,
            tc.tile_pool(name="work", bufs=2) as wpool,
            tc.tile_pool(name="psum", bufs=2, space="PSUM") as ppool,
            tc.tile_pool(name="dram", bufs=1, space="DRAM") as dpool,
        ):
            # ---- internal DRAM ----
            g1d = dpool.tile([NPAD, HID], bf, name="g1d")
            g2s = dpool.tile([PC, OUT], bf, name="g2s")
            cA, cB = ch_tiles
            g2fA = dpool.tile([N_CORES * cA * 128, OUT], bf, name="g2fA", addr_space="Shared")
            g2fB = dpool.tile([N_CORES * cB * 128, OUT], bf, name="g2fB", addr_space="Shared")

            # ---- constants ----
            w1sb = cpool.tile([128, NK1 * HID], bf, name="w1sb")
            nc.sync.dma_start(out=w1sb[:, :], in_=w1_p[:, :])
            w2sb = cpool.tile([128, NK2 * OUT], bf, name="w2sb")
            nc.sync.dma_start(out=w2sb[:, :], in_=w2_p[:, :])
            b1sb = cpool.tile([128, HID], f32, name="b1sb")
            nc.sync.dma_start(out=b1sb[:, :], in_=b1_p[:, :])
            b2sb = cpool.tile([128, OUT], f32, name="b2sb")
            nc.sync.dma_start(out=b2sb[:, :], in_=b2_p[:, :])
            iot = cpool.tile([128, 128], f32, name="iot")
            nc.sync.dma_start(out=iot[:, :], in_=iota_p[:, :])
            idn = cpool.tile([128, 128], bf, name="idn")
            nc.sync.dma_start(out=idn[:, :], in_=ident_p[:, :])
            dnv = cpool.tile([128, TT], f32, name="dnv")
            nc.sync.dma_start(out=dnv[:, :], in_=dinv_p[:, :])

            # ---- phase 1: g1 = dinv * (x @ W1), all NPAD rows ----
            CH = 4  # node tiles per x-chunk
            for ch in range(TT // CH):
                xk = []
                for k in range(NK1):
                    xt = wpool.tile([128, CH * 128], bf, tag=f"xk{k}", bufs=2)
                    nc.sync.dma_start(
                        out=xt[:, :],
                        in_=xT_p[k * 128 : (k + 1) * 128, ch * CH * 128 : (ch + 1) * CH * 128],
                    )
                    xk.append(xt)
                for sub in range(CH):
                    t = ch * CH + sub
                    ps = ppool.tile([128, HID], f32, tag="p1", bufs=2)
                    for k in range(NK1):
                        nc.tensor.matmul(
                            ps[:, :],
                            xk[k][:, sub * 128 : (sub + 1) * 128],
                            w1sb[:, k * HID : (k + 1) * HID],
                            start=(k == 0),
                            stop=(k == NK1 - 1),
                        )
                    ge = wpool.tile([128, HID], bf, tag="ge", bufs=3)
                    nc.vector.tensor_scalar(
                        ge[:, :], ps[:, :], dnv[:, t : t + 1], None, mybir.AluOpType.mult
                    )
                    nc.sync.dma_start(out=g1d[t * 128 : (t + 1) * 128, :], in_=ge[:, :])

            # ---- phase 2: layer-1 aggregation + fused mm2 for own tiles ----
            NB1 = KL1 + KH1
            for t in range(T):
                ix = wpool.tile([128, NB1 * 8], i16, tag="ix1", bufs=2)
                nc.sync.dma_start(out=ix[:, :], in_=idx1_p[t * 128 : (t + 1) * 128, :])
                dl = wpool.tile([128, NB1], f32, tag="dl1", bufs=2)
                nc.sync.dma_start(out=dl[:, :], in_=dl1_p[t * 128 : (t + 1) * 128, :])
                ml = wpool.tile([128, KL1, HID], bf, tag="ml1", bufs=2)
                nc.gpsimd.dma_gather(
                    ml[:, :, :], g1d[0:HALF, :], ix[:, : KL1 * 8],
                    KL1 * 128, KL1 * 128, HID, single_packet=False,
                )
                mh = wpool.tile([128, KH1, HID], bf, tag="mh1", bufs=2)
                nc.gpsimd.dma_gather(
                    mh[:, :, :], g1d[HALF:, :], ix[:, KL1 * 8 :],
                    KH1 * 128, KH1 * 128, HID, single_packet=False,
                )
                gs = wpool.tile([128, HID], bf, tag="gs1", bufs=2)
                nc.sync.dma_start(out=gs[:, :], in_=g1d[t * 128 : (t + 1) * 128, :])

                ps = ppool.tile([128, HID], f32, tag="p1", bufs=2)
                for b in range(NB1):
                    oh = wpool.tile([128, 128], bf, tag="oh", bufs=4)
                    nc.vector.tensor_scalar(
                        oh[:, :], iot[:, :], dl[:, b : b + 1], None,
                        mybir.AluOpType.is_equal,
                    )
                    src = ml[:, b, :] if b < KL1 else mh[:, b - KL1, :]
                    nc.tensor.matmul(ps[:, :], oh[:, :], src, start=(b == 0), stop=False)
                # self-loop: psum += I @ gs
                nc.tensor.matmul(ps[:, :], idn[:, :], gs[:, :], start=False, stop=True)

                t2 = wpool.tile([128, HID], f32, tag="t2", bufs=2)
                nc.vector.tensor_scalar(
                    t2[:, :], ps[:, :], dnv[:, t : t + 1], None, mybir.AluOpType.mult
                )
                t3 = wpool.tile([128, HID], f32, tag="t3", bufs=2)
                nc.vector.tensor_tensor(t3[:, :], t2[:, :], b1sb[:, :], mybir.AluOpType.add)
                a1 = wpool.tile([128, HID], bf, tag="a1", bufs=2)
                nc.scalar.activation(a1[:, :], t3[:, :], mybir.ActivationFunctionType.Relu)

                ps2 = ppool.tile([128, OUT], f32, tag="p2", bufs=2)
                for k in range(NK2):
                    pT = ppool.tile([128, 128], bf, tag="pT", bufs=2)
                    nc.tensor.transpose(pT[:, :], a1[:, k * 128 : (k + 1) * 128], idn[:, :])
                    aT = wpool.tile([128, 128], bf, tag="aT", bufs=2)
                    nc.vector.tensor_copy(aT[:, :], pT[:, :])
                    nc.tensor.matmul(
                        ps2[:, :], aT[:, :], w2sb[:, k * OUT : (k + 1) * OUT],
                        start=(k == 0), stop=(k == NK2 - 1),
                    )
                g2e = wpool.tile([128, OUT], bf, tag="g2e", bufs=3)
                nc.vector.tensor_scalar(
                    g2e[:, :], ps2[:, :], dnv[:, t : t + 1], None, mybir.AluOpType.mult
                )
                nc.sync.dma_start(out=g2s[t * 128 : (t + 1) * 128, :], in_=g2e[:, :])

            # ---- phase 2.5: chunked AllGather of g2 slices (2 chunks = lo/hi) ----
            nc.gpsimd.collective_compute(
                "AllGather",
                mybir.AluOpType.bypass,
                ins=[g2s[0 : cA * 128, :].opt()],
                outs=[g2fA[:, :].opt()],
                replica_groups=[list(range(N_CORES))],
            )
            nc.gpsimd.collective_compute(
                "AllGather",
                mybir.AluOpType.bypass,
                ins=[g2s[cA * 128 :, :].opt()],
                outs=[g2fB[:, :].opt()],
                replica_groups=[list(range(N_CORES))],
            )

            # ---- phase 3: layer-2 aggregation -> output ----
            NB2 = KL2 + KH2
            for t in range(T):
                ix2 = wpool.tile([128, NB2 * 8], i16, tag="ix2", bufs=2)
                nc.sync.dma_start(out=ix2[:, :], in_=idx2_p[t * 128 : (t + 1) * 128, :])
                d2 = wpool.tile([128, NB2], f32, tag="dl2", bufs=2)
                nc.sync.dma_start(out=d2[:, :], in_=dl2_p[t * 128 : (t + 1) * 128, :])
                ml2 = wpool.tile([128, KL2, OUT], bf, tag="ml2", bufs=2)
                nc.gpsimd.dma_gather(
                    ml2[:, :, :], g2fA[:, :], ix2[:, : KL2 * 8],
                    KL2 * 128, KL2 * 128, OUT, single_packet=False,
                )
                mh2 = wpool.tile([128, KH2, OUT], bf, tag="mh2", bufs=2)
                nc.gpsimd.dma_gather(
                    mh2[:, :, :], g2fB[:, :], ix2[:, KL2 * 8 :],
                    KH2 * 128, KH2 * 128, OUT, single_packet=False,
                )
                gs2 = wpool.tile([128, OUT], bf, tag="gs2", bufs=2)
                nc.sync.dma_start(out=gs2[:, :], in_=g2s[t * 128 : (t + 1) * 128, :])

                ps3 = ppool.tile([128, OUT], f32, tag="p2", bufs=2)
                for b in range(NB2):
                    oh2 = wpool.tile([128, 128], bf, tag="oh", bufs=4)
                    nc.vector.tensor_scalar(
                        oh2[:, :], iot[:, :], d2[:, b : b + 1], None,
                        mybir.AluOpType.is_equal,
                    )
                    src = ml2[:, b, :] if b < KL2 else mh2[:, b - KL2, :]
                    nc.tensor.matmul(ps3[:, :], oh2[:, :], src, start=(b == 0), stop=False)
                nc.tensor.matmul(ps3[:, :], idn[:, :], gs2[:, :], start=False, stop=True)

                u2 = wpool.tile([128, OUT], f32, tag="u2", bufs=2)
                nc.vector.tensor_scalar(
                    u2[:, :], ps3[:, :], dnv[:, t : t + 1], None, mybir.AluOpType.mult
                )
                of = wpool.tile([128, OUT], f32, tag="of", bufs=3)
                nc.vector.tensor_tensor(of[:, :], u2[:, :], b2sb[:, :], mybir.AluOpType.add)
                nc.sync.dma_start(out=out_p[t * 128 : (t + 1) * 128, :], in_=of[:, :])

    nc.compile()
    return nc


def _preprocess(x, edge_index, W1, b1, W2, b2, cfg):
    N, E = cfg["N"], cfg["E"]
    IN, HID, OUT = cfg["IN"], cfg["HID"], cfg["OUT"]
    T = cfg["T"]
    PC = T * 128
    NPAD = PC * N_CORES
    TT = T * N_CORES
    HALF = NPAD // 2

    src = np.asarray(edge_index[0], dtype=np.int64)
    dst = np.asarray(edge_index[1], dtype=np.int64)

    indeg = np.bincount(dst, minlength=N)
    deg = indeg.astype(np.float32) + 1.0
    dinv = 1.0 / np.sqrt(deg)

    # ---- balanced node -> (tile, slot) assignment (LPT greedy) ----
    order = np.argsort(-indeg, kind="stable")
    heap = [(0, t, 0) for t in range(TT)]  # (load, tile, used)
    heapq.heapify(heap)
    row_of_node = np.empty(N, dtype=np.int64)
    for n in order:
        load, t, used = heapq.heappop(heap)
        row_of_node[n] = t * 128 + used
        used += 1
        if used < 128 and t * 128 + used < NPAD:
            heapq.heappush(heap, (load + int(indeg[n]), t, used))
    # note: NPAD - N pad slots simply remain unassigned

    node_of_row = np.full(NPAD, -1, dtype=np.int64)
    node_of_row[row_of_node] = np.arange(N)

    # ---- layer-2 chunk-major row mapping ----
    cA = (T + 1) // 2
    ch_tiles = [cA, T - cA]
    ch_off = np.concatenate([[0], np.cumsum(ch_tiles)])  # tile offsets within core
    blk_off = np.concatenate([[0], np.cumsum([N_CORES * c * 128 for c in ch_tiles])])
    SPLIT2 = int(blk_off[1])  # chunk A rows

    rows = np.arange(NPAD)
    r_core = rows // PC
    r_toff = (rows % PC) // 128
    r_slot = rows % 128
    r_chunk = np.searchsorted(ch_off, r_toff, side="right") - 1
    row2_of_row = (
        blk_off[r_chunk]
        + r_core * np.array(ch_tiles)[r_chunk] * 128
        + (r_toff - ch_off[r_chunk]) * 128
        + r_slot
    )

    # ---- per-edge quantities ----
    srow = row_of_node[src]
    drow = row_of_node[dst]
    e_core = drow // PC
    e_toff = (drow % PC) // 128
    e_slot = drow % 128
    srot = (srow - e_core * PC) % NPAD
    lo1 = srot < HALF
    val1 = np.where(lo1, srot, srot - HALF)
    srow2 = row2_of_row[srow]
    lo2 = srow2 < SPLIT2
    val2 = np.where(lo2, srow2, srow2 - SPLIT2)

    # ---- segment counts -> KL/KH ----
    def seg_counts(lo_flag):
        key = (e_core * T + e_toff) * 2 + (~lo_flag).astype(np.int64)
        return np.bincount(key, minlength=TT * 2).reshape(TT, 2)

    cnt1 = seg_counts(lo1)
    cnt2 = seg_counts(lo2)
    KL1 = max(1, math.ceil(cnt1[:, 0].max() / 128))
    KH1 = max(1, math.ceil(cnt1[:, 1].max() / 128))
    KL2 = max(1, math.ceil(cnt2[:, 0].max() / 128))
    KH2 = max(1, math.ceil(cnt2[:, 1].max() / 128))

    # ---- build per-core edge metadata ----
    def build_meta(lo_flag, val, KL, KH, sort_extra):
        NBK = KL + KH
        idx_arr = np.zeros((N_CORES, T, 128, NBK * 8), dtype=np.int16)
        dl_arr = np.full((N_CORES, T, 128, NBK), 999.0, dtype=np.float32)
        ordk = np.lexsort((sort_extra, val, (~lo_flag).astype(np.int64), e_toff, e_core))
        sc, st, sl, sv, ss = (
            e_core[ordk], e_toff[ordk], lo_flag[ordk], val[ordk], e_slot[ordk],
        )
        # segment boundaries
        segkey = (sc * T + st) * 2 + (~sl).astype(np.int64)
        bnd = np.concatenate([[0], np.where(np.diff(segkey) != 0)[0] + 1, [len(segkey)]])
        for i in range(len(bnd) - 1):
            a, b = bnd[i], bnd[i + 1]
            k = segkey[a]
            c, t, h = k // (T * 2), (k // 2) % T, k % 2
            n = b - a
            cap = (KL if h == 0 else KH) * 128
            assert n <= cap
            li = np.zeros(cap, dtype=np.int64)
            li[:n] = sv[a:b]
            dll = np.full(cap, 999.0, dtype=np.float32)
            dll[:n] = ss[a:b]
            colbase = 0 if h == 0 else KL * 8
            nb = cap // 128
            idx_arr[c, t, :, colbase : colbase + nb * 8] = _pack_idx(li)
            bb = 0 if h == 0 else KL
            dl_arr[c, t, :, bb : bb + nb] = dll.reshape(nb, 128).T
        return idx_arr, dl_arr

    idx1, dl1 = build_meta(lo1, val1, KL1, KH1, srow)
    idx2, dl2 = build_meta(lo2, val2, KL2, KH2, srow2)

    # ---- dense host tensors ----
    xPermT = np.zeros((IN, NPAD), dtype=np.float32)
    xPermT[:, row_of_node] = np.asarray(x, dtype=np.float32).T
    dinv_row = np.zeros(NPAD, dtype=np.float32)
    dinv_row[row_of_node] = dinv

    NK1, NK2 = IN // 128, HID // 128
    w1p = (
        np.asarray(W1, np.float32).reshape(NK1, 128, HID).transpose(1, 0, 2).reshape(128, NK1 * HID).astype(BF16)
    )
    w2p = (
        np.asarray(W2, np.float32).reshape(NK2, 128, OUT).transpose(1, 0, 2).reshape(128, NK2 * OUT).astype(BF16)
    )
    b1r = np.tile(np.asarray(b1, np.float32)[None, :], (128, 1))
    b2r = np.tile(np.asarray(b2, np.float32)[None, :], (128, 1))
    iota = np.tile(np.arange(128, dtype=np.float32)[None, :], (128, 1))
    identb = np.eye(128, dtype=np.float32).astype(BF16)

    in_maps = []
    for c in range(N_CORES):
        xr = np.roll(xPermT, -c * PC, axis=1).astype(BF16)
        dr = np.roll(dinv_row, -c * PC)
        dinvT = dr.reshape(TT, 128).T.astype(np.float32).copy()
        in_maps.append(
            {
                "xT": xr,
                "w1p": w1p,
                "w2p": w2p,
                "b1r": b1r,
                "b2r": b2r,
                "iota": iota,
                "identb": identb,
                "dinvT": dinvT,
                "idx1": idx1[c].reshape(cfg["T"] * 128, -1),
                "dl1": dl1[c].reshape(cfg["T"] * 128, -1),
                "idx2": idx2[c].reshape(cfg["T"] * 128, -1),
                "dl2": dl2[c].reshape(cfg["T"] * 128, -1),
            }
        )

    meta = dict(
        KL1=KL1, KH1=KH1, KL2=KL2, KH2=KH2, ch_tiles=ch_tiles, SPLIT2=SPLIT2,
        row_of_node=row_of_node,
    )
    return in_maps, meta


def kernel(x, edge_index, W1, b1, W2, b2):
    cfg = _CFG
    N, OUT = cfg["N"], cfg["OUT"]
    PC = cfg["T"] * 128
    in_maps, meta = _preprocess(x, edge_index, W1, b1, W2, b2, cfg)
    nc = _build_nc(cfg, meta)
    import os
    if os.environ.get("GNN_SIM"):
        from concourse import bass_interp

        sim = bass_interp.MultiCoreSim(nc, N_CORES)
        for c in range(N_CORES):
            for k, v in in_maps[c].items():
                sim.cores[c].tensor(k)[:] = v
        sim.simulate()
        results = [
            {"out": np.array(sim.cores[c].tensor("out"))} for c in range(N_CORES)
        ]
    else:
        res = run_bass_kernel_spmd(nc, in_maps, core_ids=list(range(N_CORES)))
        results = res.results
    out = np.empty((N, OUT), dtype=np.float32)
    row = meta["row_of_node"]
    core = row // PC
    local = row % PC
    for c in range(N_CORES):
        m = core == c
        out[np.where(m)[0]] = results[c]["out"][local[m]]
    return out


# revision 5
# speedup vs baseline: 70.0516x; 70.0516x over previous
"""Two-layer GCN (PyG GCNConv x2 + ReLU) on 8 Trainium2 NeuronCores.

Strategy: nodes are packed into 128-slot tiles balanced by in-degree and
sharded across cores (49 tiles/core). Each core redundantly computes the
full g1 = dinv * (x @ W1) gather table from a per-core ROTATED copy of x^T
(own nodes at rows 0..6271, so all SPMD addressing is static). Aggregation
is dma_gather of source rows + one-hot (is_equal) matmuls accumulating in
PSUM; self-loops are an identity matmul. Layer-2 input g2 = dinv * (a1 @ W2)
is computed fused per tile and exchanged with a chunked AllGather.
"""

import math
import heapq

import numpy as np
import ml_dtypes

from concourse import bacc, mybir
from concourse.tile import TileContext
from concourse.bass_utils import run_bass_kernel_spmd

BF16 = ml_dtypes.bfloat16
N_CORES = 8

# cost-model predicted makespan (ns) of the last _build_nc, for diagnostics
LAST_PREDICTED_NS = None


def _capture_schedule(tc_cls):
    orig = tc_cls.schedule_and_allocate

    def patched(self, validate_deps=False):
        global LAST_PREDICTED_NS
        r = orig(self, validate_deps)
        try:
            LAST_PREDICTED_NS = int(r[1].time)
        except Exception:
            pass
        return r

    if getattr(tc_cls, "_gnn_patched", False):
        return
    tc_cls.schedule_and_allocate = patched
    tc_cls._gnn_patched = True


_capture_schedule(TileContext)

# Full-problem config. Tests may monkeypatch _CFG before calling kernel().
_CFG = dict(
    N=50000,
    E=800000,
    IN=768,
    HID=512,
    OUT=256,
    T=49,  # tiles per core
)


def _pack_idx(idx_linear):
    """[K*128] int -> [128, K*8] int16 in dma_gather layout."""
    num = idx_linear.shape[0]
    a = idx_linear.reshape(num // 16, 16).T.astype(np.int16)
    return np.tile(a, (8, 1))


def _build_nc(cfg, meta):
    IN, HID, OUT = cfg["IN"], cfg["HID"], cfg["OUT"]
    T = cfg["T"]
    PC = T * 128
    NPAD = PC * N_CORES
    TT = T * N_CORES
    HALF = NPAD // 2
    KL1, KH1, KL2, KH2 = meta["KL1"], meta["KH1"], meta["KL2"], meta["KH2"]
    NK1 = IN // 128
    NK2 = HID // 128
    # AllGather chunks (in tiles per core)
    ch_tiles = meta["ch_tiles"]

    f32 = mybir.dt.float32
    bf = mybir.dt.bfloat16
    i16 = mybir.dt.int16

    nc = bacc.Bacc(None, target_bir_lowering=False, debug=False)
    xT_p = nc.declare_dram_parameter("xT", [IN, NPAD], bf, isOutput=False)
    w1_p = nc.declare_dram_parameter("w1p", [128, NK1 * HID], bf, isOutput=False)
    w2_p = nc.declare_dram_parameter("w2p", [128, NK2 * OUT], bf, isOutput=False)
    b1_p = nc.declare_dram_parameter("b1r", [128, HID], f32, isOutput=False)
    b2_p = nc.declare_dram_parameter("b2r", [128, OUT], f32, isOutput=False)
    iota_p = nc.declare_dram_parameter("iota", [128, 128], f32, isOutput=False)
    ident_p = nc.declare_dram_parameter("identb", [128, 128], bf, isOutput=False)
    dinv_p = nc.declare_dram_parameter("dinvT", [128, TT], f32, isOutput=False)
    idx1_p = nc.declare_dram_parameter("idx1", [T * 128, (KL1 + KH1) * 8], i16, isOutput=False)
    dl1_p = nc.declare_dram_parameter("dl1", [T * 128, KL1 + KH1], f32, isOutput=False)
    idx2_p = nc.declare_dram_parameter("idx2", [T * 128, (KL2 + KH2) * 8], i16, isOutput=False)
    dl2_p = nc.declare_dram_parameter("dl2", [T * 128, KL2 + KH2], f32, isOutput=False)
    out_p = nc.declare_dram_parameter("out", [PC, OUT], f32, isOutput=True)

    with TileContext(nc) as tc:
        with (
            tc.tile_pool(name="const", bufs=1) as cpool,
            tc.tile_pool(name="work", bufs=2) as wpool,
            tc.tile_pool(name="psum", bufs=2, space="PSUM") as ppool,
            tc.tile_pool(name="dram", bufs=1, space="DRAM") as dpool,
        ):
            # ---- internal DRAM ----
            g1d = dpool.tile([NPAD, HID], bf, name="g1d")
            g2s = dpool.tile([PC, OUT], bf, name="g2s")
            cA, cB = ch_tiles
            g2fA = dpool.tile([N_CORES * cA * 128, OUT], bf, name="g2fA", addr_space="Shared")
            g2fB = dpool.tile([N_CORES * cB * 128, OUT], bf, name="g2fB", addr_space="Shared")

            # ---- constants ----
            w1sb = cpool.tile([128, NK1 * HID], bf, name="w1sb")
            nc.sync.dma_start(out=w1sb[:, :], in_=w1_p[:, :])
            w2sb = cpool.tile([128, NK2 * OUT], bf, name="w2sb")
            nc.sync.dma_start(out=w2sb[:, :], in_=w2_p[:, :])
            b1sb = cpool.tile([128, HID], f32, name="b1sb")
            nc.sync.dma_start(out=b1sb[:, :], in_=b1_p[:, :])
            b2sb = cpool.tile([128, OUT], f32, name="b2sb")
            nc.sync.dma_start(out=b2sb[:, :], in_=b2_p[:, :])
            iot = cpool.tile([128, 128], f32, name="iot")
            nc.sync.dma_start(out=iot[:, :], in_=iota_p[:, :])
            idn = cpool.tile([128, 128], bf, name="idn")
            nc.sync.dma_start(out=idn[:, :], in_=ident_p[:, :])
            dnv = cpool.tile([128, TT], f32, name="dnv")
            nc.sync.dma_start(out=dnv[:, :], in_=dinv_p[:, :])

            # ---- phase 1: g1 = dinv * (x @ W1), all NPAD rows ----
            CH = 4  # node tiles per x-chunk
            for ch in range(TT // CH):
                xk = []
                for k in range(NK1):
                    xt = wpool.tile([128, CH * 128], bf, tag=f"xk{k}", bufs=2)
                    nc.sync.dma_start(
                        out=xt[:, :],
                        in_=xT_p[k * 128 : (k + 1) * 128, ch * CH * 128 : (ch + 1) * CH * 128],
                    )
                    xk.append(xt)
                for sub in range(CH):
                    t = ch * CH + sub
                    ps = ppool.tile([128, HID], f32, tag="p1", bufs=2)
                    for k in range(NK1):
                        nc.tensor.matmul(
                            ps[:, :],
                            xk[k][:, sub * 128 : (sub + 1) * 128],
                            w1sb[:, k * HID : (k + 1) * HID],
                            start=(k == 0),
                            stop=(k == NK1 - 1),
                        )
                    ge = wpool.tile([128, HID], bf, tag="ge", bufs=3)
                    nc.vector.tensor_scalar(
                        ge[:, :], ps[:, :], dnv[:, t : t + 1], None, mybir.AluOpType.mult
                    )
                    nc.sync.dma_start(out=g1d[t * 128 : (t + 1) * 128, :], in_=ge[:, :])

            # ---- phase 2: layer-1 aggregation + fused mm2 for own tiles ----
            NB1 = KL1 + KH1
            for t in range(T):
                ix = wpool.tile([128, NB1 * 8], i16, tag="ix1", bufs=2)
                nc.sync.dma_start(out=ix[:, :], in_=idx1_p[t * 128 : (t + 1) * 128, :])
                dl = wpool.tile([128, NB1], f32, tag="dl1", bufs=2)
                nc.sync.dma_start(out=dl[:, :], in_=dl1_p[t * 128 : (t + 1) * 128, :])
                ml = wpool.tile([128, KL1, HID], bf, tag="ml1", bufs=2)
                nc.gpsimd.dma_gather(
                    ml[:, :, :], g1d[0:HALF, :], ix[:, : KL1 * 8],
                    KL1 * 128, KL1 * 128, HID, single_packet=False,
                )
                mh = wpool.tile([128, KH1, HID], bf, tag="mh1", bufs=2)
                nc.gpsimd.dma_gather(
                    mh[:, :, :], g1d[HALF:, :], ix[:, KL1 * 8 :],
                    KH1 * 128, KH1 * 128, HID, single_packet=False,
                )
                gs = wpool.tile([128, HID], bf, tag="gs1", bufs=2)
                nc.sync.dma_start(out=gs[:, :], in_=g1d[t * 128 : (t + 1) * 128, :])

                ps = ppool.tile([128, HID], f32, tag="p1", bufs=2)
                for b in range(NB1):
                    oh = wpool.tile([128, 128], bf, tag="oh", bufs=4)
                    nc.vector.tensor_scalar(
                        oh[:, :], iot[:, :], dl[:, b : b + 1], None,
                        mybir.AluOpType.is_equal,
                    )
                    src = ml[:, b, :] if b < KL1 else mh[:, b - KL1, :]
                    nc.tensor.matmul(ps[:, :], oh[:, :], src, start=(b == 0), stop=False)
                # self-loop: psum += I @ gs
                nc.tensor.matmul(ps[:, :], idn[:, :], gs[:, :], start=False, stop=True)

                t2 = wpool.tile([128, HID], f32, tag="t2", bufs=2)
                nc.vector.tensor_scalar(
                    t2[:, :], ps[:, :], dnv[:, t : t + 1], None, mybir.AluOpType.mult
                )
                t3 = wpool.tile([128, HID], f32, tag="t3", bufs=2)
                nc.vector.tensor_tensor(t3[:, :], t2[:, :], b1sb[:, :], mybir.AluOpType.add)
                a1 = wpool.tile([128, HID], bf, tag="a1", bufs=2)
                nc.scalar.activation(a1[:, :], t3[:, :], mybir.ActivationFunctionType.Relu)

                ps2 = ppool.tile([128, OUT], f32, tag="p2", bufs=2)
                for k in range(NK2):
                    pT = ppool.tile([128, 128], bf, tag="pT", bufs=2)
                    nc.tensor.transpose(pT[:, :], a1[:, k * 128 : (k + 1) * 128], idn[:, :])
                    aT = wpool.tile([128, 128], bf, tag="aT", bufs=2)
                    nc.vector.tensor_copy(aT[:, :], pT[:, :])
                    nc.tensor.matmul(
                        ps2[:, :], aT[:, :], w2sb[:, k * OUT : (k + 1) * OUT],
                        start=(k == 0), stop=(k == NK2 - 1),
                    )
                g2e = wpool.tile([128, OUT], bf, tag="g2e", bufs=3)
                nc.vector.tensor_scalar(
                    g2e[:, :], ps2[:, :], dnv[:, t : t + 1], None, mybir.AluOpType.mult
                )
                nc.sync.dma_start(out=g2s[t * 128 : (t + 1) * 128, :], in_=g2e[:, :])

            # ---- phase 2.5: chunked AllGather of g2 slices (2 chunks = lo/hi) ----
            nc.gpsimd.collective_compute(
                "AllGather",
                mybir.AluOpType.bypass,
                ins=[g2s[0 : cA * 128, :].opt()],
                outs=[g2fA[:, :].opt()],
                replica_groups=[list(range(N_CORES))],
            )
            nc.gpsimd.collective_compute(
                "AllGather",
                mybir.AluOpType.bypass,
                ins=[g2s[cA * 128 :, :].opt()],
                outs=[g2fB[:, :].opt()],
                replica_groups=[list(range(N_CORES))],
            )

            # ---- phase 3: layer-2 aggregation -> output ----
            NB2 = KL2 + KH2
            for t in range(T):
                ix2 = wpool.tile([128, NB2 * 8], i16, tag="ix2", bufs=2)
                nc.sync.dma_start(out=ix2[:, :], in_=idx2_p[t * 128 : (t + 1) * 128, :])
                d2 = wpool.tile([128, NB2], f32, tag="dl2", bufs=2)
                nc.sync.dma_start(out=d2[:, :], in_=dl2_p[t * 128 : (t + 1) * 128, :])
                ml2 = wpool.tile([128, KL2, OUT], bf, tag="ml2", bufs=2)
                nc.gpsimd.dma_gather(
                    ml2[:, :, :], g2fA[:, :], ix2[:, : KL2 * 8],
                    KL2 * 128, KL2 * 128, OUT, single_packet=False,
                )
                mh2 = wpool.tile([128, KH2, OUT], bf, tag="mh2", bufs=2)
                nc.gpsimd.dma_gather(
                    mh2[:, :, :], g2fB[:, :], ix2[:, KL2 * 8 :],
                    KH2 * 128, KH2 * 128, OUT, single_packet=False,
                )
                gs2 = wpool.tile([128, OUT], bf, tag="gs2", bufs=2)
                nc.sync.dma_start(out=gs2[:, :], in_=g2s[t * 128 : (t + 1) * 128, :])

                ps3 = ppool.tile([128, OUT], f32, tag="p2", bufs=2)
                for b in range(NB2):
                    oh2 = wpool.tile([128, 128], bf, tag="oh", bufs=4)
                    nc.vector.tensor_scalar(
                        oh2[:, :], iot[:, :], d2[:, b : b + 1], None,
                        mybir.AluOpType.is_equal,
                    )
                    src = ml2[:, b, :] if b < KL2 else mh2[:, b - KL2, :]
                    nc.tensor.matmul(ps3[:, :], oh2[:, :], src, start=(b == 0), stop=False)
                nc.tensor.matmul(ps3[:, :], idn[:, :], gs2[:, :], start=False, stop=True)

                u2 = wpool.tile([128, OUT], f32, tag="u2", bufs=2)
                nc.vector.tensor_scalar(
                    u2[:, :], ps3[:, :], dnv[:, t : t + 1], None, mybir.AluOpType.mult
                )
                of = wpool.tile([128, OUT], f32, tag="of", bufs=3)
                nc.vector.tensor_tensor(of[:, :], u2[:, :], b2sb[:, :], mybir.AluOpType.add)
                nc.sync.dma_start(out=out_p[t * 128 : (t + 1) * 128, :], in_=of[:, :])

    nc.compile()
    return nc


def _preprocess(x, edge_index, W1, b1, W2, b2, cfg):
    N, E = cfg["N"], cfg["E"]
    IN, HID, OUT = cfg["IN"], cfg["HID"], cfg["OUT"]
    T = cfg["T"]
    PC = T * 128
    NPAD = PC * N_CORES
    TT = T * N_CORES
    HALF = NPAD // 2

    src = np.asarray(edge_index[0], dtype=np.int64)
    dst = np.asarray(edge_index[1], dtype=np.int64)

    indeg = np.bincount(dst, minlength=N)
    deg = indeg.astype(np.float32) + 1.0
    dinv = 1.0 / np.sqrt(deg)

    # ---- balanced node -> (tile, slot) assignment (LPT greedy) ----
    order = np.argsort(-indeg, kind="stable")
    heap = [(0, t, 0) for t in range(TT)]  # (load, tile, used)
    heapq.heapify(heap)
    row_of_node = np.empty(N, dtype=np.int64)
    for n in order:
        load, t, used = heapq.heappop(heap)
        row_of_node[n] = t * 128 + used
        used += 1
        if used < 128 and t * 128 + used < NPAD:
            heapq.heappush(heap, (load + int(indeg[n]), t, used))
    # note: NPAD - N pad slots simply remain unassigned

    node_of_row = np.full(NPAD, -1, dtype=np.int64)
    node_of_row[row_of_node] = np.arange(N)

    # ---- layer-2 chunk-major row mapping ----
    cA = (T + 1) // 2
    ch_tiles = [cA, T - cA]
    ch_off = np.concatenate([[0], np.cumsum(ch_tiles)])  # tile offsets within core
    blk_off = np.concatenate([[0], np.cumsum([N_CORES * c * 128 for c in ch_tiles])])
    SPLIT2 = int(blk_off[1])  # chunk A rows

    rows = np.arange(NPAD)
    r_core = rows // PC
    r_toff = (rows % PC) // 128
    r_slot = rows % 128
    r_chunk = np.searchsorted(ch_off, r_toff, side="right") - 1
    row2_of_row = (
        blk_off[r_chunk]
        + r_core * np.array(ch_tiles)[r_chunk] * 128
        + (r_toff - ch_off[r_chunk]) * 128
        + r_slot
    )

    # ---- per-edge quantities ----
    srow = row_of_node[src]
    drow = row_of_node[dst]
    e_core = drow // PC
    e_toff = (drow % PC) // 128
    e_slot = drow % 128
    srot = (srow - e_core * PC) % NPAD
    lo1 = srot < HALF
    val1 = np.where(lo1, srot, srot - HALF)
    srow2 = row2_of_row[srow]
    lo2 = srow2 < SPLIT2
    val2 = np.where(lo2, srow2, srow2 - SPLIT2)

    # ---- segment counts -> KL/KH ----
    def seg_counts(lo_flag):
        key = (e_core * T + e_toff) * 2 + (~lo_flag).astype(np.int64)
        return np.bincount(key, minlength=TT * 2).reshape(TT, 2)

    cnt1 = seg_counts(lo1)
    cnt2 = seg_counts(lo2)
    KL1 = max(1, math.ceil(cnt1[:, 0].max() / 128))
    KH1 = max(1, math.ceil(cnt1[:, 1].max() / 128))
    KL2 = max(1, math.ceil(cnt2[:, 0].max() / 128))
    KH2 = max(1, math.ceil(cnt2[:, 1].max() / 128))

    # ---- build per-core edge metadata ----
    def build_meta(lo_flag, val, KL, KH, sort_extra):
        NBK = KL + KH
        idx_arr = np.zeros((N_CORES, T, 128, NBK * 8), dtype=np.int16)
        dl_arr = np.full((N_CORES, T, 128, NBK), 999.0, dtype=np.float32)
        ordk = np.lexsort((sort_extra, val, (~lo_flag).astype(np.int64), e_toff, e_core))
        sc, st, sl, sv, ss = (
            e_core[ordk], e_toff[ordk], lo_flag[ordk], val[ordk], e_slot[ordk],
        )
        # segment boundaries
        segkey = (sc * T + st) * 2 + (~sl).astype(np.int64)
        bnd = np.concatenate([[0], np.where(np.diff(segkey) != 0)[0] + 1, [len(segkey)]])
        for i in range(len(bnd) - 1):
            a, b = bnd[i], bnd[i + 1]
            k = segkey[a]
            c, t, h = k // (T * 2), (k // 2) % T, k % 2
            n = b - a
            cap = (KL if h == 0 else KH) * 128
            assert n <= cap
            li = np.zeros(cap, dtype=np.int64)
            li[:n] = sv[a:b]
            dll = np.full(cap, 999.0, dtype=np.float32)
            dll[:n] = ss[a:b]
            colbase = 0 if h == 0 else KL * 8
            nb = cap // 128
            idx_arr[c, t, :, colbase : colbase + nb * 8] = _pack_idx(li)
            bb = 0 if h == 0 else KL
            dl_arr[c, t, :, bb : bb + nb] = dll.reshape(nb, 128).T
        return idx_arr, dl_arr

    idx1, dl1 = build_meta(lo1, val1, KL1, KH1, srow)
    idx2, dl2 = build_meta(lo2, val2, KL2, KH2, srow2)

    # ---- dense host tensors ----
    xPermT = np.zeros((IN, NPAD), dtype=np.float32)
    xPermT[:, row_of_node] = np.asarray(x, dtype=np.float32).T
    dinv_row = np.zeros(NPAD, dtype=np.float32)
    dinv_row[row_of_node] = dinv

    NK1, NK2 = IN // 128, HID // 128
    w1p = (
        np.asarray(W1, np.float32).reshape(NK1, 128, HID).transpose(1, 0, 2).reshape(128, NK1 * HID).astype(BF16)
    )
    w2p = (
        np.asarray(W2, np.float32).reshape(NK2, 128, OUT).transpose(1, 0, 2).reshape(128, NK2 * OUT).astype(BF16)
    )
    b1r = np.tile(np.asarray(b1, np.float32)[None, :], (128, 1))
    b2r = np.tile(np.asarray(b2, np.float32)[None, :], (128, 1))
    iota = np.tile(np.arange(128, dtype=np.float32)[None, :], (128, 1))
    identb = np.eye(128, dtype=np.float32).astype(BF16)

    in_maps = []
    for c in range(N_CORES):
        xr = np.roll(xPermT, -c * PC, axis=1).astype(BF16)
        dr = np.roll(dinv_row, -c * PC)
        dinvT = dr.reshape(TT, 128).T.astype(np.float32).copy()
        in_maps.append(
            {
                "xT": xr,
                "w1p": w1p,
                "w2p": w2p,
                "b1r": b1r,
                "b2r": b2r,
                "iota": iota,
                "identb": identb,
                "dinvT": dinvT,
                "idx1": idx1[c].reshape(cfg["T"] * 128, -1),
                "dl1": dl1[c].reshape(cfg["T"] * 128, -1),
                "idx2": idx2[c].reshape(cfg["T"] * 128, -1),
                "dl2": dl2[c].reshape(cfg["T"] * 128, -1),
            }
        )

    meta = dict(
        KL1=KL1, KH1=KH1, KL2=KL2, KH2=KH2, ch_tiles=ch_tiles, SPLIT2=SPLIT2,
        row_of_node=row_of_node,
    )
    return in_maps, meta


def kernel(x, edge_index, W1, b1, W2, b2):
    cfg = _CFG
    N, OUT = cfg["N"], cfg["OUT"]
    PC = cfg["T"] * 128
    in_maps, meta = _preprocess(x, edge_index, W1, b1, W2, b2, cfg)
    nc = _build_nc(cfg, meta)
    import os
    if os.environ.get("GNN_SIM"):
        from concourse import bass_interp

        sim = bass_interp.MultiCoreSim(nc, N_CORES)
        for c in range(N_CORES):
            for k, v in in_maps[c].items():
                sim.cores[c].tensor(k)[:] = v
        sim.simulate()
        results = [
            {"out": np.array(sim.cores[c].tensor("out"))} for c in range(N_CORES)
        ]
    else:
        res = run_bass_kernel_spmd(nc, in_maps, core_ids=list(range(N_CORES)))
        results = res.results
    out = np.empty((N, OUT), dtype=np.float32)
    row = meta["row_of_node"]
    core = row // PC
    local = row % PC
    for c in range(N_CORES):
        m = core == c
        out[np.where(m)[0]] = results[c]["out"][local[m]]
    return out
